# revision 9
# baseline (speedup 1.0000x reference)
"""Causal multi-head attention (B=2, T=2048, C=1024, H=16, D=64) on 8 TRN2 cores.

Sharding: core c -> batch b = c//4, head-group hg = c%4 (4 heads/core).
Each core computes its 4 heads' attention and a partial output projection
(contraction over its 256 feature columns of W_proj); the host sums the 4
partials per batch.

All device-side layouts are transposed on host so the kernel needs no
on-device transposes:
  xT  [C, T]   = x[b].T
  wqT/wkT/wvT [C, 256] = W_{q,k,v}[rows].T
  wpT [256, C] = W_proj[:, rows].T
Attention math per head (D=64):
  qT/kT [64, T] = (wqT chunk).T @ xT            (PE, f32r)
  v     [T, 64] = (xT chunk).T @ wvT            (+ ones col -> row sums)
  sT    [s, t]  = kT.T @ qT                     (K=64)
  pT            = exp(sT/8)  (no max-subtraction needed: |scores/8| < ~7)
  causal: tri-mask on diagonal 128-blocks only; lower kb blocks restricted
  oT_aug[65, t] = v_aug.T @ pT    (row 64 = softmax sums, free)
  oT_norm       = oT * (1/sums)   (reciprocal + DMA partition-broadcast + mul)
  y     [t, p]  = oT.T @ wpT      (partial over this core's 256 features)
"""

import sys

sys.path.insert(0, "/opt/trn_rl_repo")

import numpy as np

import concourse.bass as bass  # noqa: E402
import concourse.mybir as mybir  # noqa: E402
import concourse.tile as tile  # noqa: E402
from concourse import bacc  # noqa: E402
from concourse.bass_utils import run_bass_kernel_spmd  # noqa: E402

F32 = mybir.dt.float32
F32R = mybir.dt.float32r

T = 2048
C = 1024
HL = 4  # heads per core
D = 64
HD = HL * D  # 256 local feature dim
TC = 512  # t-chunk for attention
NTC = T // TC  # 4
SB = 128  # s block
NSB = T // SB  # 16
N_CORES = 8

# matmul input dtype: float32r streams 1 row/cycle at N>=256 (4x faster than
# plain float32) with near-fp32 accuracy (fp32 accumulate).
MM_DT = F32R


def _mm(ap):
    return ap


def _build_program():
    nc = bacc.Bacc("TRN2", target_bir_lowering=False, debug=False)

    xT_d = nc.dram_tensor("xT", [C, T], MM_DT, kind="ExternalInput")
    wqT_d = nc.dram_tensor("wqT", [C, HD], MM_DT, kind="ExternalInput")
    wkT_d = nc.dram_tensor("wkT", [C, HD], MM_DT, kind="ExternalInput")
    wvT_d = nc.dram_tensor("wvT", [C, HD], MM_DT, kind="ExternalInput")
    wpT_d = nc.dram_tensor("wpT", [HD, C], MM_DT, kind="ExternalInput")
    mask_d = nc.dram_tensor("mask", [SB, SB], F32, kind="ExternalInput")
    ones_d = nc.dram_tensor("ones", [SB, 1], MM_DT, kind="ExternalInput")
    y_d = nc.dram_tensor("y", [T, C], F32, kind="ExternalOutput")

    NKC = C // SB  # 8 contraction chunks of 128

    with tile.TileContext(nc) as tc:
        with (
            tc.tile_pool(name="persist", bufs=1) as persist,
            tc.tile_pool(name="pt", bufs=3) as pt_pool,
            tc.tile_pool(name="norm", bufs=2) as norm_pool,
            tc.tile_pool(name="ps_mm", bufs=2, space="PSUM") as ps_mm,
            tc.tile_pool(name="ps_s", bufs=2, space="PSUM") as ps_s,
            tc.tile_pool(name="ps_o", bufs=2, space="PSUM") as ps_o,
            tc.tile_pool(name="ps_y", bufs=2, space="PSUM") as ps_y,
        ):
            # ---- persistent SBUF tiles (packed [128, nchunks*width]) ----
            xT_sb = persist.tile([SB, NKC * T], MM_DT)  # chunk n: [:, n*T:(n+1)*T]
            wq_sb = persist.tile([SB, NKC * HD], MM_DT)
            wk_sb = persist.tile([SB, NKC * HD], MM_DT)
            wv_sb = persist.tile([SB, NKC * HD], MM_DT)
            wp_sb = persist.tile([SB, (HD // SB) * C], MM_DT)  # 2 chunks of [128, 1024]
            qT_sb = persist.tile([SB, 2 * T], MM_DT)  # grp g: heads 2g,2g+1
            kT_sb = persist.tile([SB, 2 * T], MM_DT)
            v_sb = persist.tile([SB, NSB * HL * (D + 1)], MM_DT)  # s-chunk n: n*260
            oT_sb = persist.tile([SB, 2 * T], MM_DT)
            mask_sb = persist.tile([SB, SB], F32)

            def load_packed(sb, dram_ap, width):
                n = dram_ap.shape[0] // SB
                nc.sync.dma_start(
                    sb[:].rearrange("p (n w) -> p n w", n=n),
                    dram_ap.rearrange("(n p) w -> p n w", p=SB),
                )

            load_packed(xT_sb, xT_d.ap(), T)
            load_packed(wq_sb, wqT_d.ap(), HD)
            load_packed(wk_sb, wkT_d.ap(), HD)
            load_packed(wv_sb, wvT_d.ap(), HD)
            load_packed(wp_sb, wpT_d.ap(), C)
            nc.sync.dma_start(mask_sb[:], mask_d.ap())

            # ones columns of v_aug (col 64 of each head's 65-col group)
            v_ones = v_sb[:].rearrange("p (k d) -> p k d", k=NSB * HL)[:, :, D : D + 1]
            ones_src = ones_d.ap().unsqueeze(1).to_broadcast((SB, NSB * HL, 1))
            nc.sync.dma_start(v_ones, ones_src)

            # ---- phase 2: QKV projections ----
            for dst, w_sb, eng in ((qT_sb, wq_sb, "s"), (kT_sb, wk_sb, "v")):
                for g in range(2):  # partition group (2 heads each)
                    for j in range(NTC):
                        ps = ps_mm.tile([SB, TC], F32, tag="mm")
                        for n in range(NKC):
                            nc.tensor.matmul(
                                ps[:],
                                _mm(w_sb[:, n * HD + g * SB : n * HD + (g + 1) * SB]),
                                _mm(xT_sb[:, n * T + j * TC : n * T + (j + 1) * TC]),
                                start=(n == 0),
                                stop=(n == NKC - 1),
                            )
                        dst_ap = dst[:, g * T + j * TC : g * T + (j + 1) * TC]
                        if eng == "s":
                            nc.scalar.copy(dst_ap, ps[:])
                        else:
                            nc.vector.tensor_copy(dst_ap, ps[:])

            for n in range(NSB):  # v: out [128 s, 256 d]
                ps = ps_mm.tile([SB, HD], F32, tag="mm")
                for m in range(NKC):
                    nc.tensor.matmul(
                        ps[:],
                        _mm(xT_sb[:, m * T + n * SB : m * T + (n + 1) * SB]),
                        _mm(wv_sb[:, m * HD : (m + 1) * HD]),
                        start=(m == 0),
                        stop=(m == NKC - 1),
                    )
                dst = v_sb[:, n * HL * (D + 1) : (n + 1) * HL * (D + 1)].rearrange(
                    "p (h d) -> p h d", h=HL
                )[:, :, 0:D]
                src = ps[:].rearrange("p (h d) -> p h d", h=HL)
                nc.scalar.copy(dst, src)

            # ---- phase 3: attention per head ----
            for h in range(HL):
                hp = D * (h % 2)  # partition base within group
                hg = h // 2  # column group
                for j in range(NTC):
                    po = ps_o.tile([D + 1, TC], F32, tag="o")
                    last_kb = HL * j + 3
                    for kb in range(last_kb + 1):
                        tstart = max(0, (kb - HL * j) * SB)
                        pss = ps_s.tile([SB, TC], F32, tag="s")
                        nc.tensor.matmul(
                            pss[:, tstart:],
                            _mm(
                                kT_sb[
                                    hp : hp + D,
                                    hg * T + kb * SB : hg * T + (kb + 1) * SB,
                                ]
                            ),
                            _mm(
                                qT_sb[
                                    hp : hp + D,
                                    hg * T + j * TC + tstart : hg * T + (j + 1) * TC,
                                ]
                            ),
                            start=True,
                            stop=True,
                        )
                        pT = pt_pool.tile([SB, TC], MM_DT, tag="pt")
                        nc.scalar.activation(
                            pT[:, tstart:],
                            pss[:, tstart:],
                            mybir.ActivationFunctionType.Exp,
                            scale=float(D) ** -0.5,
                        )
                        if kb >= HL * j:  # diagonal block: causal tri-mask
                            nc.vector.tensor_mul(
                                pT[:, tstart : tstart + SB],
                                pT[:, tstart : tstart + SB],
                                mask_sb[:],
                            )
                        nc.tensor.matmul(
                            po[:, tstart:],
                            _mm(
                                v_sb[
                                    :,
                                    kb * HL * (D + 1)
                                    + h * (D + 1) : kb * HL * (D + 1)
                                    + (h + 1) * (D + 1),
                                ]
                            ),
                            _mm(pT[:, tstart:]),
                            start=(kb == 0),
                            stop=(kb == last_kb),
                        )
                    # normalize: oT_sb[...] = po[0:64] * (1 / po[64])
                    row = norm_pool.tile([1, TC], F32, tag="row")
                    nc.vector.reciprocal(row[:], po[D : D + 1, :])
                    bc = norm_pool.tile([D, TC], F32, tag="bc")
                    nc.gpsimd.partition_broadcast(bc[:], row[:])
                    nc.vector.tensor_mul(
                        oT_sb[hp : hp + D, hg * T + j * TC : hg * T + (j + 1) * TC],
                        po[0:D, :],
                        bc[:],
                    )

            # ---- phase 4: output projection ----
            for i in range(NSB):
                for half in range(2):
                    ps = ps_y.tile([SB, TC], F32, tag="y")
                    for g in range(2):
                        nc.tensor.matmul(
                            ps[:],
                            _mm(oT_sb[:, g * T + i * SB : g * T + (i + 1) * SB]),
                            _mm(wp_sb[:, g * C + half * TC : g * C + (half + 1) * TC]),
                            start=(g == 0),
                            stop=(g == 1),
                        )
                    y_sb = pt_pool.tile([SB, TC], F32, tag="ysb")
                    if (i + half) % 2 == 0:
                        nc.vector.tensor_copy(y_sb[:], ps[:])
                    else:
                        nc.scalar.copy(y_sb[:], ps[:])
                    nc.sync.dma_start(
                        y_d.ap()[i * SB : (i + 1) * SB, half * TC : (half + 1) * TC],
                        y_sb[:],
                    )

    nc.compile()
    return nc


_NC_CACHE = None


def _get_program():
    global _NC_CACHE
    if _NC_CACHE is None:
        _NC_CACHE = _build_program()
    return _NC_CACHE


def _make_in_maps(x, W_k, W_q, W_v, W_proj):
    mask = np.triu(np.ones((SB, SB), dtype=np.float32))  # mask[s,t]=1 iff s<=t
    in_maps = []
    for c in range(N_CORES):
        b, hg = c // 4, c % 4
        rows = slice(hg * HD, (hg + 1) * HD)
        in_maps.append(
            {
                "xT": np.ascontiguousarray(x[b].T).astype(np.float32),
                "wqT": np.ascontiguousarray(W_q[rows].T).astype(np.float32),
                "wkT": np.ascontiguousarray(W_k[rows].T).astype(np.float32),
                "wvT": np.ascontiguousarray(W_v[rows].T).astype(np.float32),
                "wpT": np.ascontiguousarray(W_proj[:, rows].T).astype(np.float32),
                "mask": mask,
                "ones": np.ones((SB, 1), dtype=np.float32),
            }
        )
    return in_maps


def _run(x, W_k, W_q, W_v, W_proj, **spmd_kwargs):
    nc = _get_program()
    in_maps = _make_in_maps(x, W_k, W_q, W_v, W_proj)
    res = run_bass_kernel_spmd(nc, in_maps, list(range(N_CORES)), **spmd_kwargs)
    ys = [res.results[c]["y"] for c in range(N_CORES)]
    out = np.stack(
        [
            ys[0] + ys[1] + ys[2] + ys[3],
            ys[4] + ys[5] + ys[6] + ys[7],
        ]
    ).astype(np.float32)
    return out, res


def kernel(x, W_k, W_q, W_v, W_proj):
    out, _ = _run(
        np.asarray(x), np.asarray(W_k), np.asarray(W_q), np.asarray(W_v),
        np.asarray(W_proj),
    )
    return out
